# revision 35
# baseline (speedup 1.0000x reference)
"""Causal self-attention (lit-gpt style, partial RoPE) on 8 Trainium2 NeuronCores.

Sharding: tensor-parallel over heads. Each core owns 4 of the 32 heads
end-to-end (QKV projection, RoPE, causal SDPA, and the K-split slice of the
output projection). Each core emits a partial (T, 4096) output; the host sums
the 8 partials (mathematically the all-reduce) and applies the bias folds.

Device-side compute dtype: bf16 matmul inputs with fp32 PSUM accumulation.

Model shapes (hardcoded): B=1, T=2048, C=4096, H=32, D=128, R=32 (rope),
rope base 10000.

v3 scheduling notes (vs the 625us v2):
 - QKV of chunk c+1 streams THROUGH chunk c's SDPA as PE filler: the
   exp->mask->PV latency chains hide under the projection matmul ocean, so
   the per-tile and head-boundary bubbles of v2 (~35us) disappear.
 - Causal diagonal tiles are column-trimmed: QK/exp/rowsum/PV only touch the
   un-masked query range; the rowsum/PV accumulations carry per-column-range
   stop flags. One [128,128] lower-tri mask (DVE multiply) replaces v2's four
   512-wide masks and the additive-mask matmuls.
 - PSUM = 8 banks tagged g*3 (QKV groups of 3) / s*2 / o / d / pp. Phase 0
   alternates the g-ring with the idle s/o/d slots for double-buffered
   groups; the last phase rotates proj accumulators through the idle g-ring.
 - Chunk 3's projection splits per head-pair: the h01 partial drips into
   chunk 3's own SDPA, the h23 partial lands in a second DRAM output that the
   host folds into the all-reduce. Tail shrinks from ~145 matmuls to 64.
 - Output staging rows are [128, 2048] (4KB DMA lines) on the gpsimd queue:
   4x fewer out-DMAs, and the Sync queue only carries input streams.
"""

import sys
from contextlib import ExitStack

sys.path.insert(0, "/opt/trn_rl_repo")

import numpy as np
import ml_dtypes

import concourse.bass as bass
import concourse.bacc as bacc
import concourse.tile as tile
from concourse import mybir
from concourse import bass_utils

BF16 = ml_dtypes.bfloat16

T = 2048
C = 4096
H = 32
D = 128
R = 32
ROPE_BASE = 10000.0
N_CORES = 8
H_LOC = H // N_CORES          # 4 heads per core
KT = C // 128                 # 32 contraction tiles
KB = KT // 4                  # 8 batched-DMA groups of 4 k-tiles
NCH = T // 512                # 4 token chunks of 512
FQK = 2 * H_LOC               # 8 q/k feature tiles: f=2h -> q_h, f=2h+1 -> k_h
SCALE = 1.0 / float(np.sqrt(D))

# set by test.py to capture an NTFF profile; harness leaves False
TRACE = False
LAST_EXEC_NS = None
LAST_RESULTS = None

_CACHE = {}


def _build_program(bias_zero):
    """Build + compile the single-program SPMD Bass module (same code on all
    8 cores; per-core weights arrive via in_maps)."""
    nc = bacc.Bacc(
        "TRN2",
        target_bir_lowering=False,
        debug=False,
        enable_asserts=False,
        num_devices=N_CORES,
    )
    bf = mybir.dt.bfloat16
    f32 = mybir.dt.float32

    # [p, kt, t] layouts so one DMA covers several k-tiles
    xT_d = nc.dram_tensor("xT", (128, KT, T), bf, kind="ExternalInput").ap()
    wqkT_d = nc.dram_tensor("wqkT", (128, KT, FQK * 128), bf, kind="ExternalInput").ap()
    wvT_d = nc.dram_tensor("wvT", (128, KT, H_LOC * 128), bf, kind="ExternalInput").ap()
    wpT_d = nc.dram_tensor("wpT", (128, H_LOC, C), bf, kind="ExternalInput").ap()
    bqk_d = nc.dram_tensor("bqk", (128, FQK), f32, kind="ExternalInput").ap()
    cos_d = nc.dram_tensor("cosP", (R, T), bf, kind="ExternalInput").ap()
    sin_d = nc.dram_tensor("sinP", (R, T), bf, kind="ExternalInput").ap()
    out_d = nc.dram_tensor("out", (T, C), bf, kind="ExternalOutput").ap()
    out2_d = nc.dram_tensor("out2", (512, C), bf, kind="ExternalOutput").ap()

    with tile.TileContext(nc) as tc:
        _emit(nc, tc, xT_d, wqkT_d, wvT_d, wpT_d, bqk_d, cos_d, sin_d,
              out_d, out2_d, bias_zero)

    nc.compile()
    return nc


def _emit(nc, tc, xT_d, wqkT_d, wvT_d, wpT_d, bqk_d, cos_d, sin_d,
          out_d, out2_d, bias_zero):
    bf = mybir.dt.bfloat16
    f32 = mybir.dt.float32

    # ---- persistent SBUF tensors ----------------------------------------
    frees = []

    def single(shape, dtype, name):
        t, fr = tc.tile(shape, dtype, name=name)
        frees.append(fr)
        return t

    qkT = single([128, FQK, T], bf, "qkT")          # Q^T/K^T: [d, f, t]
    vN = single([128, T // 128, H_LOC * 128], bf, "vN")  # V: [t%128, t//128, dv]
    yT = single([128, H_LOC, T], bf, "yT")          # normalized O^T per head
    ones = single([128, 128], bf, "ones")
    nc.vector.memset(ones, 1.0)
    bqk_sb = single([128, FQK], f32, "bqk_sb")
    if not bias_zero:
        nc.gpsimd.dma_start(out=bqk_sb, in_=bqk_d)
    cos_sb = single([R, T], bf, "cos_sb")
    sin_sb = single([R, T], bf, "sin_sb")
    nc.gpsimd.dma_start(out=cos_sb, in_=cos_d)
    nc.gpsimd.dma_start(out=sin_sb, in_=sin_d)
    # v weights are chunk-independent: keep resident instead of re-streaming
    wv_res = single([128, KT, 512], bf, "wv_res")

    with ExitStack() as ctx:
        xpool = ctx.enter_context(tc.tile_pool(name="xp", bufs=9))
        wqpool = ctx.enter_context(tc.tile_pool(name="wq", bufs=4))
        attpool = ctx.enter_context(tc.tile_pool(name="att", bufs=3))
        ropepool = ctx.enter_context(tc.tile_pool(name="rope", bufs=2))
        recippool = ctx.enter_context(tc.tile_pool(name="recip", bufs=1))
        rowpool = ctx.enter_context(tc.tile_pool(name="row", bufs=3))
        wppool = ctx.enter_context(tc.tile_pool(name="wp", bufs=8))
        psum = ctx.enter_context(tc.tile_pool(name="psum", bufs=1, space="PSUM"))

        def ps(tag, name):
            bufs = {"g": 3, "s": 2, "o": 1, "d": 1, "pp": 1}[tag]
            return psum.tile([128, 512], f32, name=name, tag=tag, bufs=bufs)

        # PE warm-up: throwaway matmuls while the first weight/activation
        # DMAs are in flight, to lift the PE HAM clock gate before real work.
        warm = ps("pp", "warm")
        for _ in range(44):
            nc.tensor.matmul(warm[:, 0:128], lhsT=ones, rhs=ones,
                             start=True, stop=True)

        # round-robin eviction engine: keeps both ACT and DVE FIFOs short
        ev_idx = [0]

        def evict(dst, src):
            ev_idx[0] += 1
            if ev_idx[0] % 2 == 0:
                nc.vector.tensor_copy(dst, src)
            else:
                nc.scalar.copy(dst, src)

        def emit_rope(f, c):
            # q'[0:16]  = q[0:16]*cos - q[16:32]*sin
            # q'[16:32] = q[16:32]*cos + q[0:16]*sin
            # sin_sb rows 0..15 hold -sin, rows 16..31 hold +sin (host-folded).
            cs = slice(c * 512, (c + 1) * 512)
            rows = qkT[0:R, f, cs]
            swap = ropepool.tile([R, 512], bf, name=f"swap{f}_{c}", tag="swap")
            nc.sync.dma_start(out=swap[0:16, :], in_=qkT[16:32, f, cs])
            nc.sync.dma_start(out=swap[16:32, :], in_=qkT[0:16, f, cs])
            t1 = ropepool.tile([R, 512], bf, name=f"t1_{f}_{c}", tag="t1")
            nc.vector.tensor_mul(t1, swap, sin_sb[:, cs])
            nc.vector.tensor_mul(rows, rows, cos_sb[:, cs])
            nc.vector.tensor_add(rows, rows, t1)

        # rope triples are DVE bursts: queue them and drip one per interleave
        # step so they never clog the DVE right when SDPA masks/normalizes
        # need it. SDPA flushes its own head's features defensively.
        rope_q = []

        def rope_drip(n=1):
            for _ in range(min(n, len(rope_q))):
                c, f = rope_q.pop(0)
                emit_rope(f, c)

        def rope_flush(c, fs):
            for f in fs:
                if (c, f) in rope_q:
                    rope_q.remove((c, f))
                    emit_rope(f, c)

        # next chunk's first x batch, prefetched near the end of the
        # previous phase so phase boundaries never wait on its DMA
        x_pre = {}

        def prefetch_x(c):
            if c < NCH and c not in x_pre:
                xb = xpool.tile([128, 4, 512], bf, name=f"xpre{c}", tag="x")
                nc.sync.dma_start(
                    out=xb, in_=xT_d[:, 0:4, c * 512:(c + 1) * 512])
                x_pre[c] = xb

        # ---- QKV projection stream (one chunk) --------------------------
        # 12 PSUM accumulators per chunk (8 qk features + 4 v token tiles) in
        # four groups of 3. Phase 0 alternates the g-ring with the s/o/d
        # slots (idle there) so consecutive groups double-buffer.
        x_tiles = {}

        def qkv_stream(c):
            # V first keeps the t=0 DMA burst to x+wv only (wq starts with
            # G2 once the all-cores HBM crunch eases); rope jobs drip into
            # the next phase via rope_q
            groups = [
                [("v", 0), ("v", 1), ("v", 2)],
                [("v", 3), ("f", 0), ("f", 1)],
                [("f", 2), ("f", 3), ("f", 4)],
                [("f", 5), ("f", 6), ("f", 7)],
            ]
            for gi, members in enumerate(groups):
                if c == 0 and gi % 2 == 1:
                    tags = ["s", "o", "d"]
                else:
                    tags = ["g", "g", "g"]
                pss = [
                    psum.tile([128, 512], f32, name=f"qkv{c}_{gi}_{i}",
                              tag=tags[i], bufs={"g": 3, "s": 2, "o": 1,
                                                 "d": 1}[tags[i]])
                    for i in range(3)
                ]
                fs = [m[1] for m in members if m[0] == "f"]
                for kb in range(KB):
                    if gi == 0:
                        if kb == 0 and c in x_pre:
                            x_tiles[kb] = x_pre.pop(c)
                        else:
                            xb = xpool.tile([128, 4, 512], bf,
                                            name=f"x{c}_{kb}", tag="x")
                            nc.sync.dma_start(
                                out=xb,
                                in_=xT_d[:, kb * 4:(kb + 1) * 4,
                                         c * 512:(c + 1) * 512],
                            )
                            x_tiles[kb] = xb
                        if c == 0:
                            # first batches on gpsimd: x0 and wv0 then stream
                            # in parallel through the t=0 HBM crunch
                            weng = nc.gpsimd if kb < 2 else nc.sync
                            weng.dma_start(
                                out=wv_res[:, kb * 4:(kb + 1) * 4, :],
                                in_=wvT_d[:, kb * 4:(kb + 1) * 4, :],
                            )
                    xb = x_tiles[kb]
                    wb = None
                    if fs:
                        f_lo, f_hi = fs[0], fs[-1] + 1
                        wb = wqpool.tile(
                            [128, 4, (f_hi - f_lo) * 128], bf,
                            name=f"w{c}_{gi}_{kb}", tag="wq",
                        )
                        wsrc = wqkT_d[:, kb * 4:(kb + 1) * 4,
                                      f_lo * 128:f_hi * 128]
                        nc.sync.dma_start(out=wb, in_=wsrc)
                    for k4 in range(4):
                        kt = kb * 4 + k4
                        for mi, (kind, idx) in enumerate(members):
                            if kind == "v":
                                nc.tensor.matmul(
                                    pss[mi],
                                    lhsT=xb[:, k4, idx * 128:(idx + 1) * 128],
                                    rhs=wv_res[:, kt, :],
                                    start=(kt == 0),
                                    stop=(kt == KT - 1),
                                )
                            else:
                                fo = idx - fs[0]
                                nc.tensor.matmul(
                                    pss[mi],
                                    lhsT=wb[:, k4, fo * 128:(fo + 1) * 128],
                                    rhs=xb[:, k4, :],
                                    start=(kt == 0),
                                    stop=(kt == KT - 1),
                                )
                    yield
                # evictions + rope; always on ACT so the DVE stays clear
                # for the rope/mask/normalize chains
                for mi, (kind, idx) in enumerate(members):
                    if kind == "v":
                        nc.scalar.copy(vN[:, c * 4 + idx, :], pss[mi])
                    else:
                        dst = qkT[:, idx, c * 512:(c + 1) * 512]
                        if not bias_zero:
                            nc.vector.tensor_scalar_add(
                                dst, pss[mi], bqk_sb[:, idx:idx + 1])
                        else:
                            nc.scalar.copy(dst, pss[mi])
                for kind, idx in members:
                    if kind == "f":
                        if c == 0:
                            # phase 0 has no SDPA competing for the DVE
                            emit_rope(idx, c)
                        else:
                            rope_q.append((c, idx))

        # ---- causal SDPA stream (one chunk, single-head, trimmed) --------
        def sdpa_stream(s):
            njt = 4 * (s + 1)
            for h in range(H_LOC):
                rope_flush(s, (2 * h, 2 * h + 1))
                o_t = ps("o", f"o{s}_{h}")
                d_t = ps("d", f"d{s}_{h}")
                for jt in range(njt):
                    r = jt - 4 * s
                    lo = 128 * r if r > 0 else 0
                    s_t = ps("s", f"s{s}_{h}_{jt}")
                    nc.tensor.matmul(
                        s_t[:, lo:512],
                        lhsT=qkT[:, 2 * h + 1, jt * 128:(jt + 1) * 128],
                        rhs=qkT[:, 2 * h, s * 512 + lo:(s + 1) * 512],
                        start=True,
                        stop=True,
                    )
                    att = attpool.tile([128, 512], bf,
                                       name=f"att{s}_{h}_{jt}", tag="att")
                    nc.scalar.activation(
                        out=att[:, lo:512],
                        in_=s_t[:, lo:512],
                        func=mybir.ActivationFunctionType.Exp,
                        scale=SCALE,
                    )
                    first = jt == 0
                    if r >= 0:
                        # diagonal: triangular mask on the 128-wide window,
                        # then rowsum/PV with per-range stop flags (each
                        # column range's last writer is its own diagonal)
                        hi = lo + 128
                        # causal window mask on the gpsimd engine: keeps the
                        # DVE free for rope/normalize/evictions
                        nc.gpsimd.affine_select(
                            out=att[:, lo:hi],
                            in_=att[:, lo:hi],
                            compare_op=mybir.AluOpType.is_ge,
                            fill=0.0,
                            base=0,
                            # keep where (query u) - (key kk) >= 0
                            pattern=[[1, 128]],
                            channel_multiplier=-1,
                        )
                        nc.tensor.matmul(d_t[:, lo:hi], lhsT=ones,
                                         rhs=att[:, lo:hi],
                                         start=first, stop=True)
                        nc.tensor.matmul(
                            o_t[:, lo:hi],
                            lhsT=vN[:, jt, h * 128:(h + 1) * 128],
                            rhs=att[:, lo:hi],
                            start=first, stop=True)
                        if hi < 512:
                            nc.tensor.matmul(d_t[:, hi:512], lhsT=ones,
                                             rhs=att[:, hi:512],
                                             start=first, stop=False)
                            nc.tensor.matmul(
                                o_t[:, hi:512],
                                lhsT=vN[:, jt, h * 128:(h + 1) * 128],
                                rhs=att[:, hi:512],
                                start=first, stop=False)
                    else:
                        nc.tensor.matmul(d_t, lhsT=ones, rhs=att,
                                         start=first, stop=False)
                        nc.tensor.matmul(
                            o_t,
                            lhsT=vN[:, jt, h * 128:(h + 1) * 128],
                            rhs=att,
                            start=first, stop=False)
                    yield
                rec = recippool.tile([128, 512], f32, name=f"rec{s}_{h}",
                                     tag="rec")
                nc.vector.reciprocal_approx_fast(rec, d_t)
                nc.vector.tensor_mul(yT[:, h, s * 512:(s + 1) * 512], o_t, rec)

        # ---- output projection stream (one chunk) ------------------------
        # tl-major with [128, 2048] staging rows -> 4KB DMA lines on the
        # gpsimd queue. heads selects the contraction slice (chunk-3 split);
        # row_off maps token rows into dst_d (out2 holds only chunk 3).
        def proj_stream(c, heads, dst_d, row_off, ptags, wps=None,
                        eng=None, eng2=None):
            # tl-major with [128, 2048] staging rows (4KB DMA lines). wps may
            # arrive pre-populated (chunk-3 h23 reuses h01's blocks).
            if eng is None:
                eng = nc.sync
            preloaded = wps is not None and len(wps) == 8
            if wps is None:
                wps = {}
            pi = [0]

            def wp_load(nch):
                wp = wppool.tile([128, H_LOC, 512], bf,
                                 name=f"wp{c}_{nch}", tag="wp")
                nc.sync.dma_start(
                    out=wp, in_=wpT_d[:, :, nch * 512:(nch + 1) * 512])
                return wp

            if not preloaded:
                for nch in range(2):
                    if nch not in wps:
                        wps[nch] = wp_load(nch)
            for tl in range(4):
                tt = c * 4 + tl
                r0 = tt * 128 - row_off
                for half in range(2):
                    row = rowpool.tile([128, 2048], bf,
                                       name=f"row{c}_{tl}_{half}", tag="row")
                    for q in range(4):
                        nch = half * 4 + q
                        for nxt in (nch + 2, nch + 3, nch + 4):
                            if preloaded or nxt >= 8 or nxt in wps:
                                continue
                            wps[nxt] = wp_load(nxt)
                        tag = ptags[pi[0] % len(ptags)]
                        pi[0] += 1
                        pp = psum.tile(
                            [128, 512], f32, name=f"pp{c}_{nch}_{tl}",
                            tag=tag, bufs={"g": 3, "s": 2, "o": 1, "d": 1,
                                           "pp": 1}[tag])
                        for i, h in enumerate(heads):
                            nc.tensor.matmul(
                                pp,
                                lhsT=yT[:, h, tt * 128:(tt + 1) * 128],
                                rhs=wps[nch][:, h, :],
                                start=(i == 0),
                                stop=(i == len(heads) - 1),
                            )
                        evict(row[:, q * 512:(q + 1) * 512], pp)
                        yield
                    # alternate row DMAs across two queues when given a
                    # second one, so the final transfers overlap in the drain
                    reng = eng2 if (eng2 is not None and
                                    (tl * 2 + half) % 2 == 1) else eng
                    reng.dma_start(
                        out=dst_d[r0:r0 + 128,
                                  half * 2048:(half + 1) * 2048],
                        in_=row,
                    )

        def drain(gen, n=None):
            cnt = 0
            for _ in gen:
                cnt += 1
                if n is not None and cnt >= n:
                    return False
            return True

        # generators not finished by their phase's end keep pumping inside
        # the next phase's interleave loop (a serial drain burst at a phase
        # boundary leaves the PE with no filler for the latency chains)
        carry = []

        def pump(n=1):
            for g in carry[:]:
                done = False
                for _ in range(n):
                    if next(g, StopIteration) is StopIteration:
                        done = True
                        break
                if done:
                    carry.remove(g)

        # ---- phase 0: QKV(0) alone --------------------------------------
        qg = qkv_stream(0)
        for step in range(32):
            next(qg, None)
            if step == 27:
                prefetch_x(1)
        drain(qg)

        # ---- phases 1..3: QKV(c) || SDPA(c-1) || proj(c-2) ---------------
        for c in range(1, NCH):
            s = c - 1
            qg = qkv_stream(c)
            sg = sdpa_stream(s)
            pg = proj_stream(c - 2, range(H_LOC), out_d, 0, ["pp"],
                             eng=nc.gpsimd) \
                if c >= 2 else None
            n_tiles = 16 * (s + 1)
            sd_credit = 0.0
            pj_credit = 0.0
            for step in range(32):
                next(qg, None)
                rope_drip(1)
                pump(1)
                if step == 27:
                    prefetch_x(c + 1)
                sd_credit += n_tiles / 32.0
                while sd_credit >= 1.0:
                    if next(sg, StopIteration) is StopIteration:
                        sg = None
                        sd_credit = 0.0
                        break
                    sd_credit -= 1.0
                if pg is not None:
                    pj_credit += 1.0
                    while pj_credit >= 1.0:
                        if next(pg, StopIteration) is StopIteration:
                            pj_credit = 0.0
                            pg = None
                            break
                        pj_credit -= 1.0
            drain(qg)
            if sg is not None:
                carry.append(sg)
            if pg is not None:
                carry.append(pg)

        # ---- phase 4: SDPA(3) || proj(2) + proj(3,h01); tail proj(3,h23) -
        sg = sdpa_stream(NCH - 1)
        queue = [proj_stream(NCH - 2, range(H_LOC), out_d, 0, ["g"])]
        wps3 = {}

        def drip(credit):
            while credit >= 1.0 and queue:
                if next(queue[0], StopIteration) is StopIteration:
                    queue.pop(0)
                    continue
                credit -= 1.0
            return credit

        pj_credit = 0.0
        for ti in range(64):
            next(sg, None)
            rope_drip(1)
            pump(1)
            if ti == 32:
                # heads 0,1 of chunk 3 are normalized: their half of
                # chunk 3's projection can drip now
                queue.append(
                    proj_stream(NCH - 1, range(2), out_d, 0, ["g"], wps3))
            pj_credit = drip(pj_credit + 1.0)
        drain(sg)
        while carry:
            pump(4)
        while queue:
            drain(queue.pop(0))
        # tail: chunk 3's h23 partial into the second output, reusing the
        # projection weights the h01 pass just loaded
        drain(proj_stream(NCH - 1, range(2, 4), out2_d, 3 * 512,
                          ["g", "g", "g", "s", "s", "o", "d"], wps3,
                          eng2=nc.gpsimd))

    for fr in reversed(frees):
        fr()


def _rope_tables():
    theta = 1.0 / (ROPE_BASE ** (np.arange(0, R, 2, dtype=np.float64) / R))  # (16,)
    ang = np.outer(np.arange(T, dtype=np.float64), theta)  # (T, 16)
    cos = np.cos(ang).T  # (16, T)
    sin = np.sin(ang).T
    cosP = np.concatenate([cos, cos], axis=0)  # (32, T)
    sinP = np.concatenate([-sin, sin], axis=0)
    return np.ascontiguousarray(cosP).astype(BF16), np.ascontiguousarray(sinP).astype(BF16)


def _to_p_kt(a):
    """(rows, cols) -> (128, rows//128, cols): row r = [kt*128 + p]."""
    rows, cols = a.shape
    return np.ascontiguousarray(
        a.reshape(rows // 128, 128, cols).transpose(1, 0, 2)
    )


def kernel(x, w_attn, b_attn, w_proj, b_proj):
    x = np.asarray(x, dtype=np.float32)
    w_attn = np.asarray(w_attn, dtype=np.float32)
    b_attn = np.asarray(b_attn, dtype=np.float32)
    w_proj = np.asarray(w_proj, dtype=np.float32)
    b_proj = np.asarray(b_proj, dtype=np.float32)
    B = x.shape[0]
    assert (B, x.shape[1], x.shape[2]) == (1, T, C)

    bias_zero = bool(np.all(b_attn.reshape(H, 3, D)[:, :2, :] == 0.0))
    key = ("nc", bias_zero)
    if key not in _CACHE:
        _CACHE[key] = _build_program(bias_zero)
    nc = _CACHE[key]

    xT = _to_p_kt(x[0].T.astype(BF16))  # (128, 32, T)
    cosP, sinP = _rope_tables()

    # w_attn rows per head h: [q (128), k (128), v (128)] at offset h*384
    wa = w_attn.reshape(H, 3, D, C)
    ba = b_attn.reshape(H, 3, D)
    in_maps = []
    for core in range(N_CORES):
        hs = range(core * H_LOC, (core + 1) * H_LOC)
        qk_rows = np.concatenate(
            [wa[h, t] for h in hs for t in (0, 1)], axis=0
        )  # (1024, C)  order: q_h0, k_h0, q_h1, k_h1, ...
        v_rows = np.concatenate([wa[h, 2] for h in hs], axis=0)  # (512, C)
        wqkT = _to_p_kt(qk_rows.T.astype(BF16))  # (128, 32, 1024)
        wvT = _to_p_kt(v_rows.T.astype(BF16))  # (128, 32, 512)
        wpT = _to_p_kt(
            w_proj[:, core * 512:(core + 1) * 512].T.astype(BF16)
        )  # (128, 4, C)
        bqk = np.ascontiguousarray(
            np.stack([ba[h, t] for h in hs for t in (0, 1)], axis=0).T
        ).astype(np.float32)  # (128, 8)
        in_maps.append(
            dict(
                xT=xT, wqkT=wqkT, wvT=wvT, wpT=wpT, bqk=bqk,
                cosP=cosP, sinP=sinP,
            )
        )

    res = bass_utils.run_bass_kernel_spmd(
        nc, in_maps, core_ids=list(range(N_CORES)), trace=TRACE
    )
    global LAST_EXEC_NS, LAST_RESULTS
    LAST_EXEC_NS = res.exec_time_ns
    LAST_RESULTS = res

    out = np.zeros((T, C), dtype=np.float32)
    for core in range(N_CORES):
        out += res.results[core]["out"]
        out[3 * 512:] += res.results[core]["out2"]

    # bias folds: q/k biases were applied on device; the v bias adds exactly
    # b_v to every y row (softmax rows sum to 1), so it folds into the output
    # bias along with b_proj.
    b_v = ba[:, 2, :].reshape(-1)  # (4096,)
    out += (w_proj @ b_v + b_proj)[None, :]
    return out.reshape(B, T, C).astype(np.float32)
